# revision 27
# baseline (speedup 1.0000x reference)
"""BitLinear (ternary weight quantization + linear) on 8 Trainium2 NeuronCores.

Math: out = (x @ w_q.T + b) * LAYER_SCALE, where
  beta = max(mean(|W|), eps)           (global scalar over the full W)
  w_q  = clip(round(W / beta), -1, 1) * beta   (ternary: beta * {-1, 0, +1})

Device strategy (column-parallel + data-parallel):
  8 cores = 2 batch-shards (tokens) x 4 feature-shards (out_features).

All-fp8 DoubleRow contraction at the PE fp8 peak (~216ns per K=256 N=512
DR matmul — measured; the moving-operand stream of 1024 fp8/MM at
2/cycle/partition is the hard gate; LDWEIGHTS overlaps on its own port,
weight reuse via ldweights=False gives zero gain — measured).

Precision plan (slot assignment per output tile, N_MM MMs x 256 slots):
  - k-subtiles 0..C_COR-1: one MM per subtile, slots = (hi, lo) =
    (e4m3(x), e4m3(x - hi)) against the SAME ternary weight in both slots.
  - k-subtiles C_COR..15: pairs share one MM (slots = two subtiles' e4m3(x)).
  C_COR=8 -> 12 MMs/tile (vs 16 bf16), measured rel L2 err ~1.88e-2
  (gate 2e-2, deterministic inputs). Ternary quantization decisions are
  bit-identical to the reference (host-exact beta and |W| > c threshold).

vs the 386us baseline: weights are quantized and DR-packed on the HOST
(e4m3 ternary is exact), removing the on-device DVE quantization stage and
10.5MB of f32 W traffic; the output is written as bf16 (adds ~0.11% rel
err in quadrature, halves output traffic); all inputs are SBUF-resident
(x 12.6MB + W 6.3MB), DMA'd up-front across the 3 DMA-issue queues
(sync/scalar/gpsimd) in ~0.5MB pieces ordered by first use (x block 0 +
W chunk 0 interleaved, then W chunks 1-3 in thirds, then x blocks on
sync); dummy warm-up MMs run during the ~6us framework preamble + DMA
ramp to absorb the HAM clock-gate cold state; the last token block
flushes per-m-tile on sync/gpsimd so the final drain is ~2.5us.

Fixed costs measured on this stack: ~6.3us engine-barrier/act-table
preamble before any DMA issue, ~8us until first DMA bytes land, ~6us
teardown. PE floor is 1536 x 216ns = 331.8us; measured total ~361us.
Beware occasional P0 power-downclock runs (~+19% on every MM).
"""

import math
from functools import lru_cache

import ml_dtypes
import numpy as np

import concourse.bass as bass
import concourse.mybir as mybir
import concourse.tile as tile
from concourse import bacc
from concourse.bass import ts
from concourse.bass_utils import run_bass_kernel_spmd

P = 128
IN_FEATURES = 2048
OUT_FEATURES = 8192
N_TOKENS = 8192  # 4 * 2048
EPS = 1e-8
LAYER_SCALE = np.float32(1.0 / math.sqrt(IN_FEATURES))

S_WAYS = 2  # data-parallel over tokens
Q_WAYS = 4  # tensor-parallel over out_features
N_CORES = S_WAYS * Q_WAYS

K_TILES = IN_FEATURES // P       # 16 k-subtiles
C_COR = 8                        # corrected k-subtiles (hi+lo pairs)
N_FAST = K_TILES - C_COR         # subtiles at plain e4m3 (paired 2-per-MM)
assert N_FAST % 2 == 0
N_MM = C_COR + N_FAST // 2       # DoubleRow MMs per output tile

OUT_BF16 = True                  # write output as bf16 (halves out traffic)
N_WARM = 66                      # dummy warm-up MMs during startup DMA

F32 = mybir.dt.float32
BF16 = mybir.dt.bfloat16
F8E4 = mybir.dt.float8e4
DR = mybir.MatmulPerfMode.DoubleRow


@lru_cache(maxsize=4)
def build_nc(KI: int, OC: int, TC: int, TB: int = 512):
    """Per-core bass program.

    Inputs (per core; host-relaid so every DMA is one contiguous
    descriptor per partition):
      xp   [P, T_BLOCKS, N_MM, 2, TB] f8e4 : packed x slots
      wt   [P, N_CHUNKS, N_MM, 2, CHUNK] f8e4 : ternary weights, DR-packed
      bvec [OC]   f32 : bias shard, host-reordered, pre-scaled
      consts [P, 1] f32 : beta*LAYER_SCALE per partition
    Output:
      out  [OC, TC] bf16 : (x @ w_q.T)^T shard, scaled and biased
    """
    assert KI % P == 0 and OC % P == 0 and TC % TB == 0
    assert KI // P == K_TILES
    M_TILES = OC // P
    T_BLOCKS = TC // TB
    MG = min(4, M_TILES)       # m-tiles per output DMA
    CHUNK = min(512, OC)
    N_CHUNKS = OC // CHUNK
    M_PER_CHUNK = CHUNK // P
    OUT_DT = BF16 if OUT_BF16 else F32

    nc = bacc.Bacc(None, target_bir_lowering=False, name="bitlinear")

    xp = nc.dram_tensor("xp", [P, T_BLOCKS, N_MM, 2, TB], F8E4,
                        kind="ExternalInput")
    wt = nc.dram_tensor("wt", [P, N_CHUNKS, N_MM, 2, CHUNK], F8E4,
                        kind="ExternalInput")
    bvec = nc.dram_tensor("bvec", [OC], F32, kind="ExternalInput")
    consts = nc.dram_tensor("consts", [P, 1], F32, kind="ExternalInput")
    out = nc.dram_tensor("out", [OC, TC], OUT_DT, kind="ExternalOutput")

    out_r = out[:].rearrange("(g p) t -> p g t", p=P)         # [P, M_TILES, TC]

    with tile.TileContext(nc) as tc:
        with (
            tc.tile_pool(name="const", bufs=1) as cpool,
            tc.tile_pool(name="wq", bufs=1) as wqpool,
            tc.tile_pool(name="xb", bufs=1) as xbpool,
            tc.tile_pool(name="ot", bufs=5) as opool,
            tc.tile_pool(name="ps", bufs=8, space="PSUM") as pspool,
        ):
            # --- constants + PE warm-up (runs while input DMAs stream) ---
            cst = cpool.tile([P, 1], F32)
            bs = cpool.tile([P, M_TILES], F32)
            warm = cpool.tile([P, 2, TB], F8E4)
            nc.sync.dma_start(cst[:], consts[:])
            nc.sync.dma_start(bs[:], bvec[:].rearrange("(p m) -> p m", p=P))
            scl_t = cst[:, 0:1]
            nc.vector.memset(warm[:], 0)
            wps = pspool.tile([P, TB], F32, tag="ps", name="warm_ps")
            for i in range(N_WARM):
                nc.tensor.matmul(
                    wps[:], warm[:, :, 0:P], warm[:],
                    start=(i % 8 == 0),
                    stop=(i % 8 == 7 or i == N_WARM - 1),
                    perf_mode=DR,
                )

            # --- input DMAs: everything SBUF-resident. Phase A: x block 0
            # and W chunk 0 in six ~0.5MB per-k-group pieces, placed so the
            # g0..g3 x and W pieces land first on different queues. Phase B:
            # W chunks 1..3 split across scalar+gpsimd ahead of any other
            # traffic there; x blocks 1..7 stream on sync alone. ---
            xt = xbpool.tile([P, T_BLOCKS, N_MM, 2, TB], F8E4)
            wq = wqpool.tile([P, N_CHUNKS, N_MM, 2, CHUNK], F8E4)
            G3 = N_MM // 3
            # phase A (first piece on each queue, then second piece)
            nc.sync.dma_start(xt[:, 0, :G3, :, :], xp[:, 0, :G3, :, :])
            nc.scalar.dma_start(wq[:, 0, :G3, :, :], wt[:, 0, :G3, :, :])
            nc.gpsimd.dma_start(xt[:, 0, 2 * G3 :, :, :],
                                xp[:, 0, 2 * G3 :, :, :])
            nc.sync.dma_start(wq[:, 0, G3 : 2 * G3, :, :],
                              wt[:, 0, G3 : 2 * G3, :, :])
            nc.scalar.dma_start(xt[:, 0, G3 : 2 * G3, :, :],
                                xp[:, 0, G3 : 2 * G3, :, :])
            nc.gpsimd.dma_start(wq[:, 0, 2 * G3 :, :, :],
                                wt[:, 0, 2 * G3 :, :, :])
            # phase B: W chunks 1..3 in thirds across all three queues
            for c in range(1, N_CHUNKS):
                nc.sync.dma_start(wq[:, c, :G3, :, :], wt[:, c, :G3, :, :])
                nc.scalar.dma_start(wq[:, c, G3 : 2 * G3, :, :],
                                    wt[:, c, G3 : 2 * G3, :, :])
                nc.gpsimd.dma_start(wq[:, c, 2 * G3 :, :, :],
                                    wt[:, c, 2 * G3 :, :, :])
            for tb in range(1, T_BLOCKS):
                nc.sync.dma_start(xt[:, tb, :, :, :], xp[:, tb, :, :, :])

            # --- main loop: uniform DoubleRow matmuls + fused drain ---
            ot_cur = {}  # mg -> (tile, tb)

            def flush_ot(mg):
                if mg in ot_cur:
                    t, tb_prev = ot_cur.pop(mg)
                    eng = nc.scalar if mg % 2 == 0 else nc.gpsimd
                    eng.dma_start(
                        out_r[:, ts(mg, MG), ts(tb_prev, TB)], t[:]
                    )

            def mm_tile(tb, m, flush_each=False):
                c, mi = divmod(m, M_PER_CHUNK)
                ps = pspool.tile([P, TB], F32, tag="ps")
                for g in range(N_MM):
                    nc.tensor.matmul(
                        ps[:],
                        wq[:, c, g, :, ts(mi, P)],
                        xt[:, tb, g, :, :],
                        start=(g == 0),
                        stop=(g == N_MM - 1),
                        perf_mode=DR,
                    )
                mg, mgi = divmod(m, MG)
                if mgi == 0:
                    flush_ot(mg)
                    ot_tile = opool.tile(
                        [P, MG, TB], OUT_DT, tag=f"ot{mg % 2}",
                        name=f"ot{mg % 2}"
                    )
                    ot_cur[mg] = (ot_tile, tb)
                ot, _ = ot_cur[mg]
                last_tile = flush_each and m == M_TILES - 1
                if last_tile:
                    # very last tile: drain in 4 column pieces so the final
                    # ACT -> flush -> completion chain is ~128 cols, not 512
                    QT = TB // 4
                    for q4 in range(4):
                        nc.scalar.activation(
                            ot[:, mgi, ts(q4, QT)],
                            ps[:, ts(q4, QT)],
                            mybir.ActivationFunctionType.Identity,
                            bias=bs[:, m : m + 1],
                            scale=scl_t[:, 0:1],
                        )
                        eng = nc.sync if q4 % 2 == 0 else nc.gpsimd
                        eng.dma_start(
                            out_r[:, m, tb * TB + q4 * QT : tb * TB
                                  + (q4 + 1) * QT],
                            ot[:, mgi, ts(q4, QT)],
                        )
                    ot_cur.pop(mg)
                    return
                nc.scalar.activation(
                    ot[:, mgi, :],
                    ps[:],
                    mybir.ActivationFunctionType.Identity,
                    bias=bs[:, m : m + 1],
                    scale=scl_t[:, 0:1],
                )
                if flush_each:
                    # keep scalar ACT-only in the tail; stripe sync/gpsimd
                    eng = nc.sync if m % 2 == 0 else nc.gpsimd
                    eng.dma_start(out_r[:, m, ts(tb, TB)], ot[:, mgi, :])
                    if mgi == MG - 1:
                        ot_cur.pop(mg)
                elif mgi == MG - 1:
                    flush_ot(mg)

            for tb in range(T_BLOCKS):
                for m in range(M_TILES):
                    mm_tile(tb, m, flush_each=(tb == T_BLOCKS - 1))
            for mg in list(ot_cur):
                flush_ot(mg)

    nc.compile()
    return nc


def _host_beta_cut(W: np.ndarray):
    """beta exactly as the (jax) reference computes it, plus the exact fp32
    threshold c reproducing round-half-to-even of W/beta near 0.5."""
    try:
        import jax
        import jax.numpy as jnp

        cpu = jax.local_devices(backend="cpu")[0]
        with jax.default_device(cpu):
            beta = np.float32(jnp.maximum(jnp.mean(jnp.abs(jnp.asarray(W))), EPS))
    except Exception:
        beta = np.float32(max(np.abs(W).astype(np.float64).mean(), EPS))

    v = np.float32(0.5) * beta  # exact (power-of-two scale)
    assert np.float32(v / beta) <= np.float32(0.5)
    while True:
        nv = np.nextafter(v, np.float32(np.inf))
        if np.float32(nv / beta) <= np.float32(0.5):
            v = nv
        else:
            break
    return beta, v


def _pack_x(blk_T: np.ndarray, TB: int = 512) -> np.ndarray:
    """blk_T: [KI, TC] f32 -> packed [P, T_BLOCKS, N_MM, 2, TB] f8e4 with
    per-partition-contiguous token blocks (single-descriptor DMAs)."""
    KI, TC = blk_T.shape
    kb = C_COR * P
    hi = blk_T.astype(ml_dtypes.float8_e4m3fn)
    lo = (blk_T[:kb] - hi[:kb].astype(np.float32)).astype(
        ml_dtypes.float8_e4m3fn
    )
    xpair = np.empty((N_MM, 2, P, TC), dtype=ml_dtypes.float8_e4m3fn)
    xpair[:C_COR, 0] = hi[:kb].reshape(C_COR, P, TC)
    xpair[:C_COR, 1] = lo.reshape(C_COR, P, TC)
    xpair[C_COR:] = hi[kb:].reshape(N_MM - C_COR, 2, P, TC)
    # [g, s, p, (tb tbi)] -> [p, tb, g, s, tbi]
    v = xpair.reshape(N_MM, 2, P, TC // TB, TB)
    return np.ascontiguousarray(v.transpose(2, 3, 0, 1, 4))


def _pack_w(tern_T: np.ndarray, CHUNK: int = 512) -> np.ndarray:
    """tern_T: [KI, OC] f32 ternary {-1,0,+1} -> [P, N_CHUNKS, N_MM, 2,
    CHUNK] f8e4 DR slot layout (slot pairs mirror _pack_x)."""
    KI, OC = tern_T.shape
    n_ch = OC // CHUNK
    kb = C_COR * P
    t8 = tern_T.astype(ml_dtypes.float8_e4m3fn)
    wpair = np.empty((N_MM, 2, P, OC), dtype=ml_dtypes.float8_e4m3fn)
    sub = t8[:kb].reshape(C_COR, P, OC)
    wpair[:C_COR, 0] = sub
    wpair[:C_COR, 1] = sub  # hi and lo slots share the same ternary weight
    wpair[C_COR:] = t8[kb:].reshape(N_MM - C_COR, 2, P, OC)
    # [g, s, p, (c chi)] -> [p, c, g, s, chi]
    v = wpair.reshape(N_MM, 2, P, n_ch, CHUNK)
    return np.ascontiguousarray(v.transpose(2, 3, 0, 1, 4))


def kernel(x: np.ndarray, W: np.ndarray, b: np.ndarray) -> np.ndarray:
    out, _ = _run(x, W, b)
    return out


def _run(x, W, b, **spmd_kwargs):
    x = np.ascontiguousarray(np.asarray(x, dtype=np.float32))
    W = np.ascontiguousarray(np.asarray(W, dtype=np.float32))
    b = np.ascontiguousarray(np.asarray(b, dtype=np.float32))

    B, T, KI = x.shape
    OC_full, KI2 = W.shape
    assert KI == KI2 == IN_FEATURES and OC_full == OUT_FEATURES
    NT = B * T
    assert NT == N_TOKENS

    TC = NT // S_WAYS
    OC = OUT_FEATURES // Q_WAYS

    beta, c = _host_beta_cut(W)
    S = np.float32(beta * LAYER_SCALE)
    consts_a = np.ascontiguousarray(
        np.broadcast_to(np.array([S], dtype=np.float32), (P, 1))
    )

    # host ternary quantization (bit-identical decisions to the reference)
    tern = (W > c).astype(np.float32) - (W < -c).astype(np.float32)

    xf = x.reshape(NT, KI)
    xp_s = [
        _pack_x(np.ascontiguousarray(xf[s * TC : (s + 1) * TC, :].T))
        for s in range(S_WAYS)
    ]
    w_q = [
        _pack_w(np.ascontiguousarray(tern[q * OC : (q + 1) * OC, :].T))
        for q in range(Q_WAYS)
    ]
    m_tiles = OC // P
    b_scaled = (b * LAYER_SCALE).astype(np.float32)
    b_q = [
        np.ascontiguousarray(
            b_scaled[q * OC : (q + 1) * OC].reshape(m_tiles, P).T.ravel()
        )
        for q in range(Q_WAYS)
    ]

    in_maps = []
    for s in range(S_WAYS):
        for q in range(Q_WAYS):
            in_maps.append(
                {
                    "xp": xp_s[s],
                    "wt": w_q[q],
                    "bvec": b_q[q],
                    "consts": consts_a,
                }
            )

    nc = build_nc(KI, OC, TC)
    res = run_bass_kernel_spmd(nc, in_maps, core_ids=list(range(N_CORES)),
                               **spmd_kwargs)

    out_full = np.empty((NT, OUT_FEATURES), dtype=np.float32)
    for s in range(S_WAYS):
        for q in range(Q_WAYS):
            piece = res.results[s * Q_WAYS + q]["out"]  # [OC, TC]
            out_full[s * TC : (s + 1) * TC, q * OC : (q + 1) * OC] = (
                piece.T.astype(np.float32)
            )
    return out_full.reshape(B, T, OUT_FEATURES), res


# revision 28
# speedup vs baseline: 1.0041x; 1.0041x over previous
"""BitLinear (ternary weight quantization + linear) on 8 Trainium2 NeuronCores.

Math: out = (x @ w_q.T + b) * LAYER_SCALE, where
  beta = max(mean(|W|), eps)           (global scalar over the full W)
  w_q  = clip(round(W / beta), -1, 1) * beta   (ternary: beta * {-1, 0, +1})

Device strategy (column-parallel + data-parallel):
  8 cores = 2 batch-shards (tokens) x 4 feature-shards (out_features).

All-fp8 DoubleRow contraction at the PE fp8 peak (~216ns per K=256 N=512
DR matmul — measured; the moving-operand stream of 1024 fp8/MM at
2/cycle/partition is the hard gate; LDWEIGHTS overlaps on its own port,
weight reuse via ldweights=False gives zero gain — measured).

Precision plan (slot assignment per output tile, N_MM MMs x 256 slots):
  - k-subtiles 0..C_COR-1: one MM per subtile, slots = (hi, lo) =
    (e4m3(x), e4m3(x - hi)) against the SAME ternary weight in both slots.
  - k-subtiles C_COR..15: pairs share one MM (slots = two subtiles' e4m3(x)).
  C_COR=8 -> 12 MMs/tile (vs 16 bf16), measured rel L2 err ~1.88e-2
  (gate 2e-2, deterministic inputs). Ternary quantization decisions are
  bit-identical to the reference (host-exact beta and |W| > c threshold).

vs the 386us baseline: weights are quantized and DR-packed on the HOST
(e4m3 ternary is exact), removing the on-device DVE quantization stage and
10.5MB of f32 W traffic; the output is written as bf16 (adds ~0.11% rel
err in quadrature, halves output traffic); all inputs are SBUF-resident
(x 12.6MB + W 6.3MB), DMA'd up-front across the 3 DMA-issue queues
(sync/scalar/gpsimd) in ~0.5MB pieces ordered by first use (x block 0 +
W chunk 0 interleaved, then W chunks 1-3 in thirds, then x blocks on
sync); dummy warm-up MMs run during the ~6us framework preamble + DMA
ramp to absorb the HAM clock-gate cold state; the last token block
flushes per-m-tile on sync/gpsimd so the final drain is ~2.5us.

Fixed costs measured on this stack: ~6.3us engine-barrier/act-table
preamble before any DMA issue, ~8us until first DMA bytes land, ~6us
teardown. PE floor is 1536 x 216ns = 331.8us; measured total ~361us.
Beware occasional P0 power-downclock runs (~+19% on every MM).
"""

import math
from functools import lru_cache

import ml_dtypes
import numpy as np

import concourse.bass as bass
import concourse.mybir as mybir
import concourse.tile as tile
from concourse import bacc
from concourse.bass import ts
from concourse.bass_utils import run_bass_kernel_spmd

P = 128
IN_FEATURES = 2048
OUT_FEATURES = 8192
N_TOKENS = 8192  # 4 * 2048
EPS = 1e-8
LAYER_SCALE = np.float32(1.0 / math.sqrt(IN_FEATURES))

S_WAYS = 2  # data-parallel over tokens
Q_WAYS = 4  # tensor-parallel over out_features
N_CORES = S_WAYS * Q_WAYS

K_TILES = IN_FEATURES // P       # 16 k-subtiles
C_COR = 8                        # corrected k-subtiles (hi+lo pairs)
N_FAST = K_TILES - C_COR         # subtiles at plain e4m3 (paired 2-per-MM)
assert N_FAST % 2 == 0
N_MM = C_COR + N_FAST // 2       # DoubleRow MMs per output tile

OUT_BF16 = True                  # write output as bf16 (halves out traffic)
N_WARM = 66                      # dummy warm-up MMs during startup DMA

F32 = mybir.dt.float32
BF16 = mybir.dt.bfloat16
F8E4 = mybir.dt.float8e4
DR = mybir.MatmulPerfMode.DoubleRow


@lru_cache(maxsize=4)
def build_nc(KI: int, OC: int, TC: int, TB: int = 512):
    """Per-core bass program.

    Inputs (per core; host-relaid so every DMA is one contiguous
    descriptor per partition):
      xp   [P, T_BLOCKS, N_MM, 2, TB] f8e4 : packed x slots
      wt   [P, N_CHUNKS, N_MM, 2, CHUNK] f8e4 : ternary weights, DR-packed
      bvec [OC]   f32 : bias shard, host-reordered, pre-scaled
      consts [P, 1] f32 : beta*LAYER_SCALE per partition
    Output:
      out  [OC, TC] bf16 : (x @ w_q.T)^T shard, scaled and biased
    """
    assert KI % P == 0 and OC % P == 0 and TC % TB == 0
    assert KI // P == K_TILES
    M_TILES = OC // P
    T_BLOCKS = TC // TB
    MG = min(4, M_TILES)       # m-tiles per output DMA
    CHUNK = min(512, OC)
    N_CHUNKS = OC // CHUNK
    M_PER_CHUNK = CHUNK // P
    OUT_DT = BF16 if OUT_BF16 else F32

    nc = bacc.Bacc(None, target_bir_lowering=False, name="bitlinear")

    xp = nc.dram_tensor("xp", [P, T_BLOCKS, N_MM, 2, TB], F8E4,
                        kind="ExternalInput")
    wt = nc.dram_tensor("wt", [P, N_CHUNKS, N_MM, 2, CHUNK], F8E4,
                        kind="ExternalInput")
    bvec = nc.dram_tensor("bvec", [OC], F32, kind="ExternalInput")
    consts = nc.dram_tensor("consts", [P, 1], F32, kind="ExternalInput")
    out = nc.dram_tensor("out", [OC, TC], OUT_DT, kind="ExternalOutput")

    out_r = out[:].rearrange("(g p) t -> p g t", p=P)         # [P, M_TILES, TC]

    with tile.TileContext(nc) as tc:
        with (
            tc.tile_pool(name="const", bufs=1) as cpool,
            tc.tile_pool(name="wq", bufs=1) as wqpool,
            tc.tile_pool(name="xb", bufs=1) as xbpool,
            tc.tile_pool(name="ot", bufs=3) as opool,
            tc.tile_pool(name="ps", bufs=8, space="PSUM") as pspool,
        ):
            # --- constants + PE warm-up (runs while input DMAs stream) ---
            cst = cpool.tile([P, 1], F32)
            bs = cpool.tile([P, M_TILES], F32)
            warm = cpool.tile([P, 2, TB], F8E4)
            nc.sync.dma_start(cst[:], consts[:])
            nc.sync.dma_start(bs[:], bvec[:].rearrange("(p m) -> p m", p=P))
            scl_t = cst[:, 0:1]
            nc.vector.memset(warm[:], 0)
            wps = pspool.tile([P, TB], F32, tag="ps", name="warm_ps")
            for i in range(N_WARM):
                nc.tensor.matmul(
                    wps[:], warm[:, :, 0:P], warm[:],
                    start=(i % 8 == 0),
                    stop=(i % 8 == 7 or i == N_WARM - 1),
                    perf_mode=DR,
                )

            # --- input DMAs: everything SBUF-resident. Phase A: x block 0
            # and W chunk 0 in six ~0.5MB per-k-group pieces, placed so the
            # g0..g3 x and W pieces land first on different queues. Phase B:
            # W chunks 1..3 split across scalar+gpsimd ahead of any other
            # traffic there; x blocks 1..7 stream on sync alone. ---
            xt = xbpool.tile([P, T_BLOCKS, N_MM, 2, TB], F8E4)
            wq = wqpool.tile([P, N_CHUNKS, N_MM, 2, CHUNK], F8E4)
            G3 = N_MM // 3
            # phase A (first piece on each queue, then second piece)
            nc.sync.dma_start(xt[:, 0, :G3, :, :], xp[:, 0, :G3, :, :])
            nc.scalar.dma_start(wq[:, 0, :G3, :, :], wt[:, 0, :G3, :, :])
            nc.gpsimd.dma_start(xt[:, 0, 2 * G3 :, :, :],
                                xp[:, 0, 2 * G3 :, :, :])
            nc.sync.dma_start(wq[:, 0, G3 : 2 * G3, :, :],
                              wt[:, 0, G3 : 2 * G3, :, :])
            nc.scalar.dma_start(xt[:, 0, G3 : 2 * G3, :, :],
                                xp[:, 0, G3 : 2 * G3, :, :])
            nc.gpsimd.dma_start(wq[:, 0, 2 * G3 :, :, :],
                                wt[:, 0, 2 * G3 :, :, :])
            # phase B: W chunks 1..3 in thirds across all three queues
            for c in range(1, N_CHUNKS):
                nc.sync.dma_start(wq[:, c, :G3, :, :], wt[:, c, :G3, :, :])
                nc.scalar.dma_start(wq[:, c, G3 : 2 * G3, :, :],
                                    wt[:, c, G3 : 2 * G3, :, :])
                nc.gpsimd.dma_start(wq[:, c, 2 * G3 :, :, :],
                                    wt[:, c, 2 * G3 :, :, :])
            for tb in range(1, T_BLOCKS):
                nc.sync.dma_start(xt[:, tb, :, :, :], xp[:, tb, :, :, :])

            # --- main loop: uniform DoubleRow matmuls + fused drain ---
            ot_cur = {}  # mg -> (tile, tb)

            def flush_ot(mg):
                if mg in ot_cur:
                    t, tb_prev = ot_cur.pop(mg)
                    eng = nc.scalar if mg % 2 == 0 else nc.gpsimd
                    eng.dma_start(
                        out_r[:, ts(mg, MG), ts(tb_prev, TB)], t[:]
                    )

            def mm_tile(tb, m, flush_each=False):
                c, mi = divmod(m, M_PER_CHUNK)
                ps = pspool.tile([P, TB], F32, tag="ps")
                for g in range(N_MM):
                    nc.tensor.matmul(
                        ps[:],
                        wq[:, c, g, :, ts(mi, P)],
                        xt[:, tb, g, :, :],
                        start=(g == 0),
                        stop=(g == N_MM - 1),
                        perf_mode=DR,
                    )
                mg, mgi = divmod(m, MG)
                if mgi == 0:
                    flush_ot(mg)
                    ot_tile = opool.tile(
                        [P, MG, TB], OUT_DT, tag=f"ot{mg % 2}",
                        name=f"ot{mg % 2}"
                    )
                    ot_cur[mg] = (ot_tile, tb)
                ot, _ = ot_cur[mg]
                last_tile = flush_each and m == M_TILES - 1
                if last_tile:
                    # very last tile: drain in 4 column pieces so the final
                    # ACT -> flush -> completion chain is ~128 cols, not 512
                    QT = TB // 4
                    for q4 in range(4):
                        nc.scalar.activation(
                            ot[:, mgi, ts(q4, QT)],
                            ps[:, ts(q4, QT)],
                            mybir.ActivationFunctionType.Identity,
                            bias=bs[:, m : m + 1],
                            scale=scl_t[:, 0:1],
                        )
                        eng = nc.sync if q4 % 2 == 0 else nc.gpsimd
                        eng.dma_start(
                            out_r[:, m, tb * TB + q4 * QT : tb * TB
                                  + (q4 + 1) * QT],
                            ot[:, mgi, ts(q4, QT)],
                        )
                    ot_cur.pop(mg)
                    return
                nc.scalar.activation(
                    ot[:, mgi, :],
                    ps[:],
                    mybir.ActivationFunctionType.Identity,
                    bias=bs[:, m : m + 1],
                    scale=scl_t[:, 0:1],
                )
                if flush_each:
                    # keep scalar ACT-only in the tail; stripe sync/gpsimd
                    eng = nc.sync if m % 2 == 0 else nc.gpsimd
                    eng.dma_start(out_r[:, m, ts(tb, TB)], ot[:, mgi, :])
                    if mgi == MG - 1:
                        ot_cur.pop(mg)
                elif mgi == MG - 1:
                    flush_ot(mg)

            for tb in range(T_BLOCKS):
                for m in range(M_TILES):
                    mm_tile(tb, m, flush_each=(tb == T_BLOCKS - 1))
            for mg in list(ot_cur):
                flush_ot(mg)

    nc.compile()
    return nc


def _host_beta_cut(W: np.ndarray):
    """beta exactly as the (jax) reference computes it, plus the exact fp32
    threshold c reproducing round-half-to-even of W/beta near 0.5."""
    try:
        import jax
        import jax.numpy as jnp

        cpu = jax.local_devices(backend="cpu")[0]
        with jax.default_device(cpu):
            beta = np.float32(jnp.maximum(jnp.mean(jnp.abs(jnp.asarray(W))), EPS))
    except Exception:
        beta = np.float32(max(np.abs(W).astype(np.float64).mean(), EPS))

    v = np.float32(0.5) * beta  # exact (power-of-two scale)
    assert np.float32(v / beta) <= np.float32(0.5)
    while True:
        nv = np.nextafter(v, np.float32(np.inf))
        if np.float32(nv / beta) <= np.float32(0.5):
            v = nv
        else:
            break
    return beta, v


def _pack_x(blk_T: np.ndarray, TB: int = 512) -> np.ndarray:
    """blk_T: [KI, TC] f32 -> packed [P, T_BLOCKS, N_MM, 2, TB] f8e4 with
    per-partition-contiguous token blocks (single-descriptor DMAs)."""
    KI, TC = blk_T.shape
    kb = C_COR * P
    hi = blk_T.astype(ml_dtypes.float8_e4m3fn)
    lo = (blk_T[:kb] - hi[:kb].astype(np.float32)).astype(
        ml_dtypes.float8_e4m3fn
    )
    xpair = np.empty((N_MM, 2, P, TC), dtype=ml_dtypes.float8_e4m3fn)
    xpair[:C_COR, 0] = hi[:kb].reshape(C_COR, P, TC)
    xpair[:C_COR, 1] = lo.reshape(C_COR, P, TC)
    xpair[C_COR:] = hi[kb:].reshape(N_MM - C_COR, 2, P, TC)
    # [g, s, p, (tb tbi)] -> [p, tb, g, s, tbi]
    v = xpair.reshape(N_MM, 2, P, TC // TB, TB)
    return np.ascontiguousarray(v.transpose(2, 3, 0, 1, 4))


def _pack_w(tern_T: np.ndarray, CHUNK: int = 512) -> np.ndarray:
    """tern_T: [KI, OC] f32 ternary {-1,0,+1} -> [P, N_CHUNKS, N_MM, 2,
    CHUNK] f8e4 DR slot layout (slot pairs mirror _pack_x)."""
    KI, OC = tern_T.shape
    n_ch = OC // CHUNK
    kb = C_COR * P
    t8 = tern_T.astype(ml_dtypes.float8_e4m3fn)
    wpair = np.empty((N_MM, 2, P, OC), dtype=ml_dtypes.float8_e4m3fn)
    sub = t8[:kb].reshape(C_COR, P, OC)
    wpair[:C_COR, 0] = sub
    wpair[:C_COR, 1] = sub  # hi and lo slots share the same ternary weight
    wpair[C_COR:] = t8[kb:].reshape(N_MM - C_COR, 2, P, OC)
    # [g, s, p, (c chi)] -> [p, c, g, s, chi]
    v = wpair.reshape(N_MM, 2, P, n_ch, CHUNK)
    return np.ascontiguousarray(v.transpose(2, 3, 0, 1, 4))


def kernel(x: np.ndarray, W: np.ndarray, b: np.ndarray) -> np.ndarray:
    out, _ = _run(x, W, b)
    return out


def _run(x, W, b, **spmd_kwargs):
    x = np.ascontiguousarray(np.asarray(x, dtype=np.float32))
    W = np.ascontiguousarray(np.asarray(W, dtype=np.float32))
    b = np.ascontiguousarray(np.asarray(b, dtype=np.float32))

    B, T, KI = x.shape
    OC_full, KI2 = W.shape
    assert KI == KI2 == IN_FEATURES and OC_full == OUT_FEATURES
    NT = B * T
    assert NT == N_TOKENS

    TC = NT // S_WAYS
    OC = OUT_FEATURES // Q_WAYS

    beta, c = _host_beta_cut(W)
    S = np.float32(beta * LAYER_SCALE)
    consts_a = np.ascontiguousarray(
        np.broadcast_to(np.array([S], dtype=np.float32), (P, 1))
    )

    # host ternary quantization (bit-identical decisions to the reference)
    tern = (W > c).astype(np.float32) - (W < -c).astype(np.float32)

    xf = x.reshape(NT, KI)
    xp_s = [
        _pack_x(np.ascontiguousarray(xf[s * TC : (s + 1) * TC, :].T))
        for s in range(S_WAYS)
    ]
    w_q = [
        _pack_w(np.ascontiguousarray(tern[q * OC : (q + 1) * OC, :].T))
        for q in range(Q_WAYS)
    ]
    m_tiles = OC // P
    b_scaled = (b * LAYER_SCALE).astype(np.float32)
    b_q = [
        np.ascontiguousarray(
            b_scaled[q * OC : (q + 1) * OC].reshape(m_tiles, P).T.ravel()
        )
        for q in range(Q_WAYS)
    ]

    in_maps = []
    for s in range(S_WAYS):
        for q in range(Q_WAYS):
            in_maps.append(
                {
                    "xp": xp_s[s],
                    "wt": w_q[q],
                    "bvec": b_q[q],
                    "consts": consts_a,
                }
            )

    nc = build_nc(KI, OC, TC)
    res = run_bass_kernel_spmd(nc, in_maps, core_ids=list(range(N_CORES)),
                               **spmd_kwargs)

    out_full = np.empty((NT, OUT_FEATURES), dtype=np.float32)
    for s in range(S_WAYS):
        for q in range(Q_WAYS):
            piece = res.results[s * Q_WAYS + q]["out"]  # [OC, TC]
            out_full[s * TC : (s + 1) * TC, q * OC : (q + 1) * OC] = (
                piece.T.astype(np.float32)
            )
    return out_full.reshape(B, T, OUT_FEATURES), res
